# revision 12
# baseline (speedup 1.0000x reference)
"""SSIM(3x3 avg-pool) + L1 loss kernel for Trainium2, 8 NeuronCores.

loss = 0.85 * mean(clip((1 - ssim_map)/2, 0, 1)) + 0.15 * mean(|pred - target|)

Full inputs pred/target: (16, 1, 1024, 1024) f32. Data-parallel: 2 images per
core. On this execution path every instruction costs a ~flat 50-90us
regardless of size or engine (DRAM->SBUF DMA included), so the kernel
minimizes instruction count: no matmuls, no PSUM, no activation engine -
pure DVE + DMA, ~41 instructions per core.

Structure (per image pair):
  8 stripes of 128 output rows, 2 groups of 4 stripes. Host packs
  ptin[130, 2, 16, 1026] bf16: dim0 j = per-stripe image row offset j-1
  (halo rows, zeros outside the image), dim2 = group sections ordered
  [8 p-sections | 8 t-sections] (stripe-major, image-minor), zero-padded
  to 1026 cols. bf16 staging only touches the raw inputs and per-copy
  q=2pt / w=p^2+t^2 products (~0.2% input rounding, bias ~1e-4); all
  pooling accumulation and post-pool math is f32.

  Per group (P: [128, 32832] f32 accumulator tile, S: [128, 32832] bf16
  stage tile - 197KB/partition total):
    load copy A (row offset -1) -> S.[p|t], q/w -> S.[q|w] (stt + custom
    sqsum), P = copy(S) widening to f32; copy B (offset 0): load, L1 =
    |p-t| via custom absdiff accum (rows exactly disjoint across stripes),
    q/w, P += S (one double-wide mixed-dtype add); copy C (offset +1)
    likewise. P now holds the vertical 3-tap of [p|t|q|w].
    Horizontal 3-tap written compact f32: H.pt -> S-buffer (f32 instance),
    T=[A2=2XY | V=X^2+Y^2] -> P's dead pt region, H.qw -> S-buffer, then
    ONE fused rational op R=(T+81C1)*(Hqw*9 - T + 81C2) -> [n1n2|d1d2]
    in place, reciprocal, fused clip-accumulate into per-group accumulator
    columns (the /9 pool scale is folded into the constants).
  Host sums the accumulator slices.
"""

import sys

import numpy as np

sys.path.insert(0, "/opt/trn_rl_repo")

ALPHA = 0.85
BETA = 0.15
C1 = 0.01 ** 2
C2 = 0.03 ** 2

N_CORES = 8
IMG_H = 1024
IMG_W = 1024
N_IMG_PER_CORE = 2

BLK = 128                      # output rows per stripe (halo via loads)
NS = 8                         # stripes per pair (exact: 8*128 = 1024)
KG = 4                         # stripes per group
NG = NS // KG                  # 2 groups
S = IMG_W + 2                  # padded section width (1026)
NSEC = 2 * KG                  # sections per half (8)
HW_ = NSEC * S                 # half width (8208)
WID = 2 * HW_                  # [p|t] width (16416)
DW = 2 * WID                   # [p|t|q|w] width (32832)
CW = NSEC * IMG_W              # compact half width (8192)

# scaled ssim constants (pooled fields carry a 9x box-sum scale)
SC1 = 81.0 * C1
SC2 = 81.0 * C2
SXS = 9.0                      # n2*81 = 9*B(2pt) - A2' + 81C2 (d2 likewise)

# --- custom fused DVE ops ------------------------------------------------- #
_OP_SQSUM = None       # out = in0^2 + in1^2
_OP_SSIM_RAT = None    # out = (in0 + s0) * (in1*s1 - in0 + imm2)
_OP_SSIM_FINAL = None  # out = (s0 - clamp(in0*in1, s1, s0))*imm2; accum += out
_OP_ABSD = None        # out = |in0 - in1|; accum += out
_CUSTOM_OPS_OK = False


def _register_custom_ops():
    global _OP_SQSUM, _OP_SSIM_RAT, _OP_SSIM_FINAL, _OP_ABSD, _CUSTOM_OPS_OK
    if _CUSTOM_OPS_OK:
        return
    from operator import add

    import concourse.dve_ops as dv
    from concourse.dve_spec import (
        C0, C1 as KC1, C2 as KC2, AluOp, Bin, Spec, Src0, Src1, Zero,
        lower, maxx, minn, sq,
    )
    from concourse.dve_uop import DveOpSpec

    def _sqsum_ref(in0, in1, c0, c1, c2):
        return in0.astype(np.float32) ** 2 + in1.astype(np.float32) ** 2

    def _rat_ref(in0, in1, c0, c1, c2):
        a = in0.astype(np.float32)
        return (a + c0) * (in1.astype(np.float32) * c1 - a + c2)

    def _final_ref(in0, in1, c0, c1, c2):
        z = in0.astype(np.float32) * in1.astype(np.float32)
        b = ((c0 - np.clip(z, c1, c0)) * c2).astype(np.float32)
        return b, b.reshape(b.shape[0], -1).sum(axis=-1, keepdims=True)

    def _absd_ref(in0, in1, c0, c1, c2):
        b = np.abs(in0.astype(np.float32) - in1.astype(np.float32))
        return b, b.reshape(b.shape[0], -1).sum(axis=-1, keepdims=True)

    defs = [
        ("SSIM_SQSUM_ANT", Spec(body=sq(Src0) + sq(Src1), reference=_sqsum_ref)),
        ("SSIM_RAT_ANT", Spec(
            body=(Src0 + C0) * (Src1 * KC1 - Src0 + KC2), reference=_rat_ref)),
        ("SSIM_FINAL_ANT", Spec(
            body=(C0 - maxx(minn(Src0 * Src1, C0), KC1)) * KC2,
            accum=add, accum_init=Zero, reference=_final_ref)),
        ("SSIM_ABSD_ANT", Spec(
            body=Bin(AluOp.ABSOLUTE_DIFF, Src0, Src1),
            accum=add, accum_init=Zero, reference=_absd_ref)),
    ]
    made = {}
    for name, spec in defs:
        if name not in dv._SUB_OPCODE_FOR_NAME:
            stub = dv.DveOp(name, spec, subdim=False, uops_sha={})
            dv.OPS.append(stub)
            dv._SUB_OPCODE_FOR_NAME[name] = (
                dv._CUSTOM_DVE_ROW_BASE + len(dv.OPS) - 1
            )
            dv.CUSTOM_DVE_SPECS[name] = spec
        opcode = dv._SUB_OPCODE_FOR_NAME[name]
        shas = {}
        for ver in ("v3", "v4"):
            res = DveOpSpec(
                name=name, opcode=opcode, uops=lower(spec, ver=ver),
                rd1_en=dv.has_src1(spec),
            )
            shas[ver] = res.sha(ver)
        op = dv.DveOp(name, spec, subdim=False, uops_sha=shas)
        idx = next(i for i, o in enumerate(dv.OPS) if o.name == name)
        dv.OPS[idx] = op
        dv.CUSTOM_DVE_SPECS[name] = spec
        made[name] = op
    _OP_SQSUM = made["SSIM_SQSUM_ANT"]
    _OP_SSIM_RAT = made["SSIM_RAT_ANT"]
    _OP_SSIM_FINAL = made["SSIM_FINAL_ANT"]
    _OP_ABSD = made["SSIM_ABSD_ANT"]
    _CUSTOM_OPS_OK = True


def build_program(n_img, H, W, io_internal=False):
    """Per-core program for n_img (even) HxW images.

    DRAM input "ptin": [130, npairs*NG, 2*NSEC, S] bf16 (see module doc).
    Output "acc_out": [128, 8*npairs]; per pair p columns 8p+{0,1}: L1
    partials (one per group), 8p+{4,5}: ssim partials.
    """
    import concourse.bacc as bacc
    import concourse.tile as tile
    from concourse import mybir

    assert n_img % 2 == 0
    f32 = mybir.dt.float32
    bf16 = mybir.dt.bfloat16
    Alu = mybir.AluOpType
    npairs = n_img // 2

    _register_custom_ops()
    nc = bacc.Bacc("TRN2", target_bir_lowering=False, debug=False)

    io_kind = "Internal" if io_internal else "ExternalInput"
    ptin_d = nc.dram_tensor(
        "ptin", [130, npairs * NG, 2 * NSEC, S], bf16, kind=io_kind).ap()
    acc_d = nc.dram_tensor(
        "acc_out", [128, 8 * npairs], f32, kind="ExternalOutput").ap()

    with tile.TileContext(nc) as tc:
        with (
            tc.tile_pool(name="bufP", bufs=1) as poolP,
            tc.tile_pool(name="bufS", bufs=1) as poolS,
            tc.tile_pool(name="misc", bufs=1) as mpool,
        ):
            acc = mpool.tile([128, 8 * npairs], f32, tag="acc")

            for pair in range(npairs):
                gbase = pair * NG
                cbase = pair * 8
                for g in range(NG):
                    gi = gbase + g

                    def stage(off, nm, l1_col=None):
                        # load a copy into a bf16 stage tile and compute its
                        # [q|w] products; optionally the L1 accum (copy B)
                        t = poolS.tile([128, DW], bf16, tag="S", name=nm)
                        nc.sync.dma_start(
                            out=t[:, 0:WID].rearrange(
                                "p (f c) -> p f c", f=2 * NSEC, c=S),
                            in_=ptin_d[off:off + 128, gi, :, :])
                        if l1_col is not None:
                            nc.vector._custom_dve(
                                _OP_ABSD, out=t[:, WID:WID + HW_],
                                in0=t[:, 0:HW_], in1=t[:, HW_:WID],
                                accum_out=acc[:, l1_col:l1_col + 1])
                        nc.vector.scalar_tensor_tensor(
                            t[:, WID:WID + HW_], t[:, 0:HW_], 2.0,
                            t[:, HW_:WID], op0=Alu.mult, op1=Alu.mult)
                        nc.vector._custom_dve(
                            _OP_SQSUM, out=t[:, WID + HW_:DW],
                            in0=t[:, 0:HW_], in1=t[:, HW_:WID])
                        return t

                    SA = stage(0, "SA")
                    P = poolP.tile([128, DW], f32, tag="P", name="P")
                    nc.vector.tensor_copy(P[:, :], SA[:, :])
                    SB = stage(1, "SB", l1_col=cbase + g)
                    nc.vector.tensor_add(P[:, :], P[:, :], SB[:, :])
                    SC_ = stage(2, "SC")
                    nc.vector.tensor_add(P[:, :], P[:, :], SC_[:, :])
                    # P = Bv([p|t|q|w]) f32 (vertical 3-tap done)

                    # horizontal 3-tap for ALL 4 families in one pair of
                    # adds, written compact bf16 into the stage buffer
                    # (pooled values round to bf16 once; accumulation and
                    # all post math stay f32)
                    Pall = P[:, :].rearrange(
                        "p (f c) -> p f c", f=4 * NSEC, c=S)
                    Hb = poolS.tile([128, DW], bf16, tag="S", name="Hb")
                    Hc = Hb[:, 0:4 * CW].rearrange(
                        "p (f c) -> p f c", f=4 * NSEC, c=W)
                    nc.vector.tensor_add(
                        Hc, Pall[:, :, 0:W], Pall[:, :, 1:W + 1])
                    nc.vector.tensor_add(Hc, Hc, Pall[:, :, 2:W + 2])
                    # Hb compact: [X | Y | B(q) | B(w)] bf16

                    # T = [A2=2XY | V=X^2+Y^2] f32 -> P's dead [p|t] region
                    X = Hb[:, 0:CW]
                    Y = Hb[:, CW:2 * CW]
                    nc.vector.scalar_tensor_tensor(
                        P[:, 0:CW], X, 2.0, Y, op0=Alu.mult, op1=Alu.mult)
                    nc.vector._custom_dve(
                        _OP_SQSUM, out=P[:, CW:2 * CW], in0=X, in1=Y)
                    # fused rationals: R = (T+SC1)*(Hqw*SXS - T + SC2)
                    # in place -> [n1n2 | d1d2]
                    nc.vector._custom_dve(
                        _OP_SSIM_RAT, out=P[:, 0:2 * CW], in0=P[:, 0:2 * CW],
                        in1=Hb[:, 2 * CW:4 * CW], s0=SC1, s1=SXS, imm2=SC2)
                    # reciprocal of d1d2 -> P's dead qw region, then fused
                    # clip-accumulate
                    nc.vector.reciprocal_approx_fast(
                        P[:, WID:WID + CW], P[:, CW:2 * CW])
                    nc.vector._custom_dve(
                        _OP_SSIM_FINAL, out=P[:, WID + CW:WID + 2 * CW],
                        in0=P[:, 0:CW], in1=P[:, WID:WID + CW],
                        s0=1.0, s1=-1.0, imm2=0.5,
                        accum_out=acc[:, cbase + 4 + g: cbase + 5 + g])

            nc.sync.dma_start(out=acc_d[:, :], in_=acc[:, :])

    nc.compile()
    return nc


_CACHE = {}


def _get_program(n_img, H, W):
    key = (n_img, H, W)
    if key not in _CACHE:
        _CACHE[key] = build_program(n_img, H, W)
    return _CACHE[key]


def make_bmats(H):
    """Compat stub for older harnesses (no matmuls in this kernel)."""
    return np.zeros((1, 1), dtype=np.float32)


def _pack_inputs(pred, target):
    """pred/target [n_img, H, W] -> packed [130, npairs*NG, 2*NSEC, S] bf16."""
    import ml_dtypes

    n_img, H, W = pred.shape
    assert n_img % 2 == 0
    npairs = n_img // 2
    out = np.zeros((130, npairs * NG, 2 * NSEC, S), dtype=ml_dtypes.bfloat16)
    pad_h = BLK * (NS - 1) + 130
    # dram j, stripe s -> padded row index 128*s + j (j=0 -> image row -1)
    J = (BLK * np.arange(NS)[None, :] + np.arange(130)[:, None])  # [130, NS]
    for pair in range(npairs):
        fields = (pred[2 * pair], pred[2 * pair + 1],
                  target[2 * pair], target[2 * pair + 1])
        for half in range(2):  # 0: p, 1: t
            for img in range(2):
                Pimg = np.zeros((pad_h, W), dtype=np.float32)
                Pimg[1:H + 1] = fields[2 * half + img]
                R = Pimg[J]  # [130, NS, W]
                for g in range(NG):
                    for s in range(KG):
                        out[:, pair * NG + g, half * NSEC + 2 * s + img,
                            1:W + 1] = R[:, g * KG + s].astype(
                                ml_dtypes.bfloat16)
    return out


LAST_RESULTS = None


def kernel(pred, target):
    from concourse.bass_utils import run_bass_kernel_spmd

    global LAST_RESULTS

    pred = np.asarray(pred, dtype=np.float32).reshape(16, IMG_H, IMG_W)
    target = np.asarray(target, dtype=np.float32).reshape(16, IMG_H, IMG_W)

    nc = _get_program(N_IMG_PER_CORE, IMG_H, IMG_W)

    in_maps = []
    for c in range(N_CORES):
        sl = slice(c * N_IMG_PER_CORE, (c + 1) * N_IMG_PER_CORE)
        in_maps.append({"ptin": _pack_inputs(pred[sl], target[sl])})

    res = run_bass_kernel_spmd(nc, in_maps, list(range(N_CORES)))
    LAST_RESULTS = res
    npairs = N_IMG_PER_CORE // 2
    ssim_sum = 0.0
    l1_sum = 0.0
    for r in res.results:
        acc = r["acc_out"]
        for p in range(npairs):
            b = 8 * p
            l1_sum += float(acc[:, b:b + NG].sum(dtype=np.float64))
            ssim_sum += float(acc[:, b + 4:b + 4 + NG].sum(dtype=np.float64))
    n = 16.0 * IMG_H * IMG_W
    loss = ALPHA * (ssim_sum / n) + BETA * (l1_sum / n)
    return np.float32(loss)


# revision 13
# speedup vs baseline: 1.9945x; 1.9945x over previous
"""SSIM(3x3 avg-pool) + L1 loss kernel for Trainium2, 8 NeuronCores.

loss = 0.85 * mean(clip((1 - ssim_map)/2, 0, 1)) + 0.15 * mean(|pred - target|)

Full inputs pred/target: (16, 1, 1024, 1024) f32. Data-parallel: 2 images per
core. On this execution path every instruction costs a ~flat 50-90us
regardless of size or engine (DRAM->SBUF DMA included), so the kernel
minimizes instruction count: no matmuls, no PSUM, no activation engine -
pure DVE + DMA, ~41 instructions per core.

Structure (per image pair):
  8 stripes of 128 output rows, 2 groups of 4 stripes. Host packs
  ptin[130, 2, 16, 1026] bf16: dim0 j = per-stripe image row offset j-1
  (halo rows, zeros outside the image), dim2 = group sections ordered
  [8 p-sections | 8 t-sections] (stripe-major, image-minor), zero-padded
  to 1026 cols. bf16 staging only touches the raw inputs and per-copy
  q=2pt / w=p^2+t^2 products (~0.2% input rounding, bias ~1e-4); all
  pooling accumulation and post-pool math is f32.

  Per group (P: [128, 32832] f32 accumulator tile, S: [128, 32832] bf16
  stage tile - 197KB/partition total):
    load copy A (row offset -1) -> S.[p|t], q/w -> S.[q|w] (stt + custom
    sqsum), P = copy(S) widening to f32; copy B (offset 0): load, L1 =
    |p-t| via custom absdiff accum (rows exactly disjoint across stripes),
    q/w, P += S (one double-wide mixed-dtype add); copy C (offset +1)
    likewise. P now holds the vertical 3-tap of [p|t|q|w].
    Horizontal 3-tap written compact f32: H.pt -> S-buffer (f32 instance),
    T=[A2=2XY | V=X^2+Y^2] -> P's dead pt region, H.qw -> S-buffer, then
    ONE fused rational op R=(T+81C1)*(Hqw*9 - T + 81C2) -> [n1n2|d1d2]
    in place, reciprocal, fused clip-accumulate into per-group accumulator
    columns (the /9 pool scale is folded into the constants).
  Host sums the accumulator slices.
"""

import sys

import numpy as np

sys.path.insert(0, "/opt/trn_rl_repo")

ALPHA = 0.85
BETA = 0.15
C1 = 0.01 ** 2
C2 = 0.03 ** 2

N_CORES = 8
IMG_H = 1024
IMG_W = 1024
N_IMG_PER_CORE = 2

BLK = 128                      # output rows per stripe (halo via loads)
NS = 8                         # stripes per pair (exact: 8*128 = 1024)
KG = 4                         # stripes per group
NG = NS // KG                  # 2 groups
S = IMG_W + 2                  # padded section width (1026)
NSEC = 2 * KG                  # sections per half (8)
HW_ = NSEC * S                 # half width (8208)
WID = 2 * HW_                  # [p|t] width (16416)
DW = 2 * WID                   # [p|t|q|w] width (32832)
CW = NSEC * IMG_W              # compact half width (8192)

# scaled ssim constants (pooled fields carry a 9x box-sum scale)
SC1 = 81.0 * C1
SC2 = 81.0 * C2
SXS = 9.0                      # n2*81 = 9*B(2pt) - A2' + 81C2 (d2 likewise)

# --- custom fused DVE ops ------------------------------------------------- #
_OP_SQSUM = None       # out = in0^2 + in1^2
_OP_SSIM_RAT = None    # out = (in0 + s0) * (in1*s1 - in0 + imm2)
_OP_SSIM_FINAL = None  # out = (s0 - clamp(in0*in1, s1, s0))*imm2; accum += out
_OP_ABSD = None        # out = |in0 - in1|; accum += out
_CUSTOM_OPS_OK = False


def _register_custom_ops():
    global _OP_SQSUM, _OP_SSIM_RAT, _OP_SSIM_FINAL, _OP_ABSD, _CUSTOM_OPS_OK
    if _CUSTOM_OPS_OK:
        return
    from operator import add

    import concourse.dve_ops as dv
    from concourse.dve_spec import (
        C0, C1 as KC1, C2 as KC2, AluOp, Bin, Spec, Src0, Src1, Zero,
        lower, maxx, minn, sq,
    )
    from concourse.dve_uop import DveOpSpec

    def _sqsum_ref(in0, in1, c0, c1, c2):
        return in0.astype(np.float32) ** 2 + in1.astype(np.float32) ** 2

    def _rat_ref(in0, in1, c0, c1, c2):
        a = in0.astype(np.float32)
        return (a + c0) * (in1.astype(np.float32) * c1 - a + c2)

    def _final_ref(in0, in1, c0, c1, c2):
        z = in0.astype(np.float32) * in1.astype(np.float32)
        b = ((c0 - np.clip(z, c1, c0)) * c2).astype(np.float32)
        return b, b.reshape(b.shape[0], -1).sum(axis=-1, keepdims=True)

    def _absd_ref(in0, in1, c0, c1, c2):
        b = np.abs(in0.astype(np.float32) - in1.astype(np.float32))
        return b, b.reshape(b.shape[0], -1).sum(axis=-1, keepdims=True)

    defs = [
        ("SSIM_SQSUM_ANT", Spec(body=sq(Src0) + sq(Src1), reference=_sqsum_ref)),
        ("SSIM_RAT_ANT", Spec(
            body=(Src0 + C0) * (Src1 * KC1 - Src0 + KC2), reference=_rat_ref)),
        ("SSIM_FINAL_ANT", Spec(
            body=(C0 - maxx(minn(Src0 * Src1, C0), KC1)) * KC2,
            accum=add, accum_init=Zero, reference=_final_ref)),
        ("SSIM_ABSD_ANT", Spec(
            body=Bin(AluOp.ABSOLUTE_DIFF, Src0, Src1),
            accum=add, accum_init=Zero, reference=_absd_ref)),
    ]
    made = {}
    for name, spec in defs:
        if name not in dv._SUB_OPCODE_FOR_NAME:
            stub = dv.DveOp(name, spec, subdim=False, uops_sha={})
            dv.OPS.append(stub)
            dv._SUB_OPCODE_FOR_NAME[name] = (
                dv._CUSTOM_DVE_ROW_BASE + len(dv.OPS) - 1
            )
            dv.CUSTOM_DVE_SPECS[name] = spec
        opcode = dv._SUB_OPCODE_FOR_NAME[name]
        shas = {}
        for ver in ("v3", "v4"):
            res = DveOpSpec(
                name=name, opcode=opcode, uops=lower(spec, ver=ver),
                rd1_en=dv.has_src1(spec),
            )
            shas[ver] = res.sha(ver)
        op = dv.DveOp(name, spec, subdim=False, uops_sha=shas)
        idx = next(i for i, o in enumerate(dv.OPS) if o.name == name)
        dv.OPS[idx] = op
        dv.CUSTOM_DVE_SPECS[name] = spec
        made[name] = op
    _OP_SQSUM = made["SSIM_SQSUM_ANT"]
    _OP_SSIM_RAT = made["SSIM_RAT_ANT"]
    _OP_SSIM_FINAL = made["SSIM_FINAL_ANT"]
    _OP_ABSD = made["SSIM_ABSD_ANT"]
    _CUSTOM_OPS_OK = True


def build_program(n_img, H, W, io_internal=False):
    """Per-core program for n_img (even) HxW images.

    DRAM input "ptin": [130, npairs*NG, 2*NSEC, S] bf16 (see module doc).
    Output "acc_out": [128, 8*npairs]; per pair p columns 8p+{0,1}: L1
    partials (one per group), 8p+{4,5}: ssim partials.
    """
    import concourse.bacc as bacc
    import concourse.tile as tile
    from concourse import mybir

    assert n_img % 2 == 0
    f32 = mybir.dt.float32
    bf16 = mybir.dt.bfloat16
    Alu = mybir.AluOpType
    npairs = n_img // 2

    _register_custom_ops()
    nc = bacc.Bacc("TRN2", target_bir_lowering=False, debug=False)

    io_kind = "Internal" if io_internal else "ExternalInput"
    ptin_d = nc.dram_tensor(
        "ptin", [130, npairs * NG, 2 * NSEC, S], bf16, kind=io_kind).ap()
    acc_d = nc.dram_tensor(
        "acc_out", [128, 8 * npairs], f32, kind="ExternalOutput").ap()

    with tile.TileContext(nc) as tc:
        with (
            tc.tile_pool(name="bufP", bufs=1) as poolP,
            tc.tile_pool(name="bufS", bufs=1) as poolS,
            tc.tile_pool(name="misc", bufs=1) as mpool,
        ):
            acc = mpool.tile([128, 8 * npairs], f32, tag="acc")

            for pair in range(npairs):
                gbase = pair * NG
                cbase = pair * 8
                for g in range(NG):
                    gi = gbase + g

                    def stage(off, nm, l1_col=None):
                        # load a copy into a bf16 stage tile and compute its
                        # [q|w] products; optionally the L1 accum (copy B)
                        t = poolS.tile([128, DW], bf16, tag="S", name=nm)
                        nc.sync.dma_start(
                            out=t[:, 0:WID].rearrange(
                                "p (f c) -> p f c", f=2 * NSEC, c=S),
                            in_=ptin_d[off:off + 128, gi, :, :])
                        if l1_col is not None:
                            nc.vector._custom_dve(
                                _OP_ABSD, out=t[:, WID:WID + HW_],
                                in0=t[:, 0:HW_], in1=t[:, HW_:WID],
                                accum_out=acc[:, l1_col:l1_col + 1])
                        nc.vector.scalar_tensor_tensor(
                            t[:, WID:WID + HW_], t[:, 0:HW_], 2.0,
                            t[:, HW_:WID], op0=Alu.mult, op1=Alu.mult)
                        nc.vector._custom_dve(
                            _OP_SQSUM, out=t[:, WID + HW_:DW],
                            in0=t[:, 0:HW_], in1=t[:, HW_:WID])
                        return t

                    SA = stage(0, "SA")
                    P = poolP.tile([128, DW], f32, tag="P", name="P")
                    nc.vector.tensor_copy(P[:, :], SA[:, :])
                    SB = stage(1, "SB", l1_col=cbase + g)
                    nc.vector.tensor_add(P[:, :], P[:, :], SB[:, :])
                    SC_ = stage(2, "SC")
                    nc.vector.tensor_add(P[:, :], P[:, :], SC_[:, :])
                    # P = Bv([p|t|q|w]) f32 (vertical 3-tap done)

                    # horizontal 3-tap, compact f32, one family-half at a
                    # time into the stage buffer (f32 instances)
                    Ppt = P[:, 0:WID].rearrange(
                        "p (f c) -> p f c", f=2 * NSEC, c=S)
                    Hpt = poolS.tile([128, 2 * CW], f32, tag="S", name="Hpt")
                    Hc = Hpt[:, :].rearrange(
                        "p (f c) -> p f c", f=2 * NSEC, c=W)
                    nc.vector.tensor_add(
                        Hc, Ppt[:, :, 0:W], Ppt[:, :, 1:W + 1])
                    nc.vector.tensor_add(Hc, Hc, Ppt[:, :, 2:W + 2])
                    # T = [A2=2XY | V=X^2+Y^2] -> P's dead [p|t] region
                    X = Hpt[:, 0:CW]
                    Y = Hpt[:, CW:2 * CW]
                    nc.vector.scalar_tensor_tensor(
                        P[:, 0:CW], X, 2.0, Y, op0=Alu.mult, op1=Alu.mult)
                    nc.vector._custom_dve(
                        _OP_SQSUM, out=P[:, CW:2 * CW], in0=X, in1=Y)
                    # H of the [q|w] half (P's qw region still live)
                    Pqw = P[:, WID:DW].rearrange(
                        "p (f c) -> p f c", f=2 * NSEC, c=S)
                    Hqw = poolS.tile([128, 2 * CW], f32, tag="S", name="Hqw")
                    Hq = Hqw[:, :].rearrange(
                        "p (f c) -> p f c", f=2 * NSEC, c=W)
                    nc.vector.tensor_add(
                        Hq, Pqw[:, :, 0:W], Pqw[:, :, 1:W + 1])
                    nc.vector.tensor_add(Hq, Hq, Pqw[:, :, 2:W + 2])

                    # fused rationals: R = (T+SC1)*(Hqw*SXS - T + SC2)
                    # in place -> [n1n2 | d1d2]
                    nc.vector._custom_dve(
                        _OP_SSIM_RAT, out=P[:, 0:2 * CW], in0=P[:, 0:2 * CW],
                        in1=Hqw[:, 0:2 * CW], s0=SC1, s1=SXS, imm2=SC2)
                    # reciprocal of d1d2 -> P's dead qw region, then fused
                    # clip-accumulate
                    nc.vector.reciprocal_approx_fast(
                        P[:, WID:WID + CW], P[:, CW:2 * CW])
                    nc.vector._custom_dve(
                        _OP_SSIM_FINAL, out=P[:, WID + CW:WID + 2 * CW],
                        in0=P[:, 0:CW], in1=P[:, WID:WID + CW],
                        s0=1.0, s1=-1.0, imm2=0.5,
                        accum_out=acc[:, cbase + 4 + g: cbase + 5 + g])

            nc.sync.dma_start(out=acc_d[:, :], in_=acc[:, :])

    nc.compile()
    return nc


_CACHE = {}


def _get_program(n_img, H, W):
    key = (n_img, H, W)
    if key not in _CACHE:
        _CACHE[key] = build_program(n_img, H, W)
    return _CACHE[key]


def make_bmats(H):
    """Compat stub for older harnesses (no matmuls in this kernel)."""
    return np.zeros((1, 1), dtype=np.float32)


def _pack_inputs(pred, target):
    """pred/target [n_img, H, W] -> packed [130, npairs*NG, 2*NSEC, S] bf16."""
    import ml_dtypes

    n_img, H, W = pred.shape
    assert n_img % 2 == 0
    npairs = n_img // 2
    out = np.zeros((130, npairs * NG, 2 * NSEC, S), dtype=ml_dtypes.bfloat16)
    pad_h = BLK * (NS - 1) + 130
    # dram j, stripe s -> padded row index 128*s + j (j=0 -> image row -1)
    J = (BLK * np.arange(NS)[None, :] + np.arange(130)[:, None])  # [130, NS]
    for pair in range(npairs):
        fields = (pred[2 * pair], pred[2 * pair + 1],
                  target[2 * pair], target[2 * pair + 1])
        for half in range(2):  # 0: p, 1: t
            for img in range(2):
                Pimg = np.zeros((pad_h, W), dtype=np.float32)
                Pimg[1:H + 1] = fields[2 * half + img]
                R = Pimg[J]  # [130, NS, W]
                for g in range(NG):
                    for s in range(KG):
                        out[:, pair * NG + g, half * NSEC + 2 * s + img,
                            1:W + 1] = R[:, g * KG + s].astype(
                                ml_dtypes.bfloat16)
    return out


LAST_RESULTS = None


def kernel(pred, target):
    from concourse.bass_utils import run_bass_kernel_spmd

    global LAST_RESULTS

    pred = np.asarray(pred, dtype=np.float32).reshape(16, IMG_H, IMG_W)
    target = np.asarray(target, dtype=np.float32).reshape(16, IMG_H, IMG_W)

    nc = _get_program(N_IMG_PER_CORE, IMG_H, IMG_W)

    in_maps = []
    for c in range(N_CORES):
        sl = slice(c * N_IMG_PER_CORE, (c + 1) * N_IMG_PER_CORE)
        in_maps.append({"ptin": _pack_inputs(pred[sl], target[sl])})

    res = run_bass_kernel_spmd(nc, in_maps, list(range(N_CORES)))
    LAST_RESULTS = res
    npairs = N_IMG_PER_CORE // 2
    ssim_sum = 0.0
    l1_sum = 0.0
    for r in res.results:
        acc = r["acc_out"]
        for p in range(npairs):
            b = 8 * p
            l1_sum += float(acc[:, b:b + NG].sum(dtype=np.float64))
            ssim_sum += float(acc[:, b + 4:b + 4 + NG].sum(dtype=np.float64))
    n = 16.0 * IMG_H * IMG_W
    loss = ALPHA * (ssim_sum / n) + BETA * (l1_sum / n)
    return np.float32(loss)
